# revision 27
# baseline (speedup 1.0000x reference)
"""Trainium2 Bass kernel for nn_AttnBlock: dynamic-filter correlation.

Math (per sample b):
  p1[l, :]  = 11x11x64 patch of im1 at position l (l over 30x30)
  scores[p, l] = <im2 patch at p, p1[l] / max(||p1[l]||, 1e-4)>
  out[p] = max_l scores[p, l]

Decomposition used on device (per core = one (sample, p-half) pair):
  scores_un[p, l] = sum_{dy,dx} sum_c im2[c, p+(dy,dx)] * im1[c, l+(dy,dx)]
computed as 121 shift-matmuls (contraction over channels) accumulated in
PSUM, two shifts packed per matmul (K=128 bf16).  Each image is DMAed
twice: partitions 0..63 hold the raw image and partitions 64..127 hold
it shifted by one element (flat +1), which bakes the (dx, dx+1) pair
shift into the data.  The stationary operand (walrus requires a single
free dim) uses six width-30 dx-compacted copies of im2 built with one
partition-aligned DVE copy each; the dx=10 tiles (which need a +1 ROW
shift in the upper half) are built from the x tiles: cast the lower
half, then a small SBUF->SBUF DMA replicates it one row up into
partitions 64..127.  This halves HBM traffic vs also loading y-shifted
image copies.  Norms: separable 11x11 box sum of im1^2 (shift-add log
tree on DVE), one fp16 ones-matmul per l-half for the channel sum and
one for the rank-1 partition broadcast of 1/norm (fused fp32 matmuls
silently return zeros at M=1/K=1 on TRN2; single fp16 is accurate to
~5e-4 which is subdominant to the bf16 matmul error).  Scale + max-
over-l run on DVE per PSUM tile.

Sharding: 8 cores = 4 samples x 2 halves of the output-row dim (pure
data parallel, no cross-core communication).
"""

import sys

import numpy as np

if "/opt/trn_rl_repo" not in sys.path:
    sys.path.insert(0, "/opt/trn_rl_repo")

B = 4
C = 64
H = W = 40
KER = 11
HP = WP = H - KER + 1  # 30
HALF = HP // 2  # 15 output rows per core
N_CORES = 2 * B
IM2_ROWS = HALF + KER - 1  # 25 input rows needed per half

_PROGRAM = None

# (dx-pair-base, dy) cells computed as a single fp8e4 DoubleRow matmul
# (4 shifts each: dx in {2ap..2ap+3} x {dy}) instead of two bf16 matmuls.
# Each cell cuts one 450-cycle matmul per (chunk, j) but adds fp8
# quantization noise on its 4/121 of the contraction; pure-fp8 measures
# rel_err 2.2e-2 vs the 2e-2 gate, so the count below keeps the noise
# at ~2.2e-2 * sqrt(4*ncells/121) plus bf16's 1.5e-3.
FP8_CELLS = tuple((ap, dy) for ap in (0, 2) for dy in range(6))


def _build_program():
    import concourse.bass as bass
    import concourse.tile as tile
    from concourse import bacc

    mybir = bass.mybir
    dt = mybir.dt
    f32 = dt.float32
    f32r = dt.float32r
    f16 = dt.float16
    bf16 = dt.bfloat16
    from contextlib import ExitStack

    nc = bacc.Bacc(
        "TRN2",
        target_bir_lowering=False,
        debug=False,
        enable_asserts=False,
        num_devices=N_CORES,
    )
    im1_d = nc.dram_tensor("im1", [C, H, W], f32, kind="ExternalInput").ap()
    im2_d = nc.dram_tensor("im2s", [C, IM2_ROWS, W], f32, kind="ExternalInput").ap()
    out_d = nc.dram_tensor("out", [128, 4], f32, kind="ExternalOutput").ap()

    MM_DT = bf16
    f8 = dt.float8e4
    DR = mybir.MatmulPerfMode.DoubleRow
    MULT = mybir.AluOpType.mult
    MAX = mybir.AluOpType.max
    SQUARE = mybir.ActivationFunctionType.Square
    SQRT = mybir.ActivationFunctionType.Sqrt
    COPY = mybir.ActivationFunctionType.Copy

    im1_flat = im1_d.rearrange("c y x -> c (y x)").bitcast(f32r)
    im2_flat = im2_d.rearrange("c y x -> c (y x)").bitcast(f32r)
    N1 = H * W
    N2 = IM2_ROWS * W

    with tile.TileContext(nc) as tc, ExitStack() as ctx:
        consts = ctx.enter_context(tc.tile_pool(name="consts", bufs=1))
        imgs = ctx.enter_context(tc.tile_pool(name="imgs", bufs=1))
        nrm = ctx.enter_context(tc.tile_pool(name="nrm", bufs=1))
        scr = ctx.enter_context(tc.tile_pool(name="scr", bufs=2))
        reds = ctx.enter_context(tc.tile_pool(name="reds", bufs=6))
        psum = ctx.enter_context(tc.tile_pool(name="psum", bufs=8, space="PSUM"))

        # Dual-shift image tiles (contiguous DMAs): partitions 0..63 raw,
        # 64..127 flat-shifted by +1 (x+1).  The wrap columns are never
        # addressed by the operand APs below.  Kicks spread across the
        # sync/scalar/gpsimd queues so descriptors land early.
        im2x = imgs.tile([128, IM2_ROWS, W], f32r)
        im1x = imgs.tile([128, H, W], f32r)
        im1x_up = im1x[C : 2 * C].rearrange("p y x -> p (y x)")
        # im1x upper kicks go first on their engines: that group gates the
        # first matmul's rhs cast and historically finished last.
        nc.sync.dma_start(im1x_up[0 : C // 2, 0 : N1 - 1], im1_flat[0 : C // 2, 1:N1])
        nc.gpsimd.dma_start(
            im1x_up[C // 2 : C, 0 : N1 - 1], im1_flat[C // 2 : C, 1:N1]
        )
        nc.scalar.dma_start(im1x[0:C], im1_flat)
        nc.sync.dma_start(im2x[0:C], im2_flat)
        nc.gpsimd.dma_start(
            im2x[C : 2 * C].rearrange("p y x -> p (y x)")[:, 0 : N2 - 1],
            im2_flat[:, 1:N2],
        )

        ones_k = consts.tile([C, 1], f16)
        nc.vector.memset(ones_k[:], 1.0)
        ones_m = consts.tile([1, 128], f16)
        nc.vector.memset(ones_m[:], 1.0)

        # Width-30 compacted operand tiles in MM_DT: the stationary side
        # must be a single-free-dim AP, and a contiguous moving side
        # streams ~6% faster than strided reads.  One partition-aligned
        # DVE cast per tile for dx<10 (pair shift already baked into the
        # source's upper half).  The dx=10 tiles get their upper (row+1
        # shifted) half from a small SBUF->SBUF DMA of their own lower
        # half, so no y-shifted HBM loads are needed.  bi=0 casts go
        # first so the first matmul's operands are ready ASAP.
        dx_bases = [0, 2, 4, 6, 8, 10]
        im1c = []
        im2c = []
        # fp8 copies of the dx-compacted tiles for a=0..3, one contiguous
        # tile per image so a DoubleRow k-tile pair (a, a+1) is a simple
        # stride-750/1200 free dim.  They sit where the y staging tiles
        # used to, followed by a pad that restores the bf16 tiles' SBUF
        # offsets: the PE streams ~20% slower when the stationary and
        # moving operand tiles land on conflicting subbanks, so keep the
        # known-good absolute placement fixed.
        # the dual-fp8 LDWEIGHTS k-tile stride must be a multiple of 16
        # bytes, so the im2 fp8 tile pads each dx-base block from 750 to
        # 768 bytes (im1's 1200 is already a multiple of 16).
        S2 = 768
        n8 = len({ap for ap, _ in FP8_CELLS} | {ap + 1 for ap, _ in FP8_CELLS})
        if n8:
            im2c8 = imgs.tile([128, n8, S2], f8, name="im2c8")
            im1c8 = imgs.tile([128, n8, H, WP], f8, name="im1c8")
        pad_f32 = (10400 - n8 * (S2 + H * WP)) // 4
        imgs.tile([128, pad_f32], f32, name="pad")
        with tc.high_priority():
            for bi, dx in enumerate(dx_bases):
                c2 = imgs.tile([128, IM2_ROWS, WP], MM_DT, name=f"im2c_{bi}")
                c1 = imgs.tile([128, H, WP], MM_DT, name=f"im1c_{bi}")
                if dx < 10:
                    nc.vector.tensor_copy(c2[:], im2x[:, :, dx : dx + WP].bitcast(f32))
                    if bi < 2:
                        # scalar runs these concurrently with the im2c casts
                        # on DVE so the first matmuls' operands close early
                        nc.scalar.activation(
                            c1[:], im1x[:, :, dx : dx + WP].bitcast(f32), COPY
                        )
                    else:
                        nc.vector.tensor_copy(
                            c1[:], im1x[:, :, dx : dx + WP].bitcast(f32)
                        )
                else:
                    nc.vector.tensor_copy(
                        c2[0:C], im2x[0:C, :, dx : dx + WP].bitcast(f32)
                    )
                    nc.vector.tensor_copy(
                        c1[0:C], im1x[0:C, :, dx : dx + WP].bitcast(f32)
                    )
                    nc.gpsimd.dma_start(
                        c2[C : 2 * C, 0 : IM2_ROWS - 1, :], c2[0:C, 1:IM2_ROWS, :]
                    )
                    nc.gpsimd.dma_start(c1[C : 2 * C, 0 : H - 1, :], c1[0:C, 1:H, :])
                im2c.append(c2)
                im1c.append(c1)
                # fp8 copies trail the bf16 ones they duplicate; they are
                # only consumed after each group's ~37 bf16 matmuls.
                if n8 and bi < n8:
                    nc.vector.tensor_copy(
                        im2c8[:, bi, 0 : IM2_ROWS * WP].rearrange(
                            "p (y x) -> p y x", y=IM2_ROWS
                        ),
                        im2x[:, :, dx : dx + WP].bitcast(f32),
                    )
                    nc.scalar.activation(
                        im1c8[:, bi], im1x[:, :, dx : dx + WP].bitcast(f32), COPY
                    )

        # ---- norm DVE chain: separable 11x11 box sum of im1^2 over (y, x).
        # Shift-add log tree: widths 1->2->4->8->11.
        sq = nrm.tile([C, H, W], f32)
        nc.scalar.activation(sq[:], im1x[0:C].bitcast(f32), SQUARE)

        t2 = nrm.tile([C, H, W - 1], f32)
        nc.vector.tensor_add(t2[:], sq[:, :, 0 : W - 1], sq[:, :, 1:W])
        t4 = nrm.tile([C, H, W - 3], f32)
        nc.vector.tensor_add(t4[:], t2[:, :, 0 : W - 3], t2[:, :, 2 : W - 1])
        t8 = nrm.tile([C, H, W - 7], f32)
        nc.vector.tensor_add(t8[:], t4[:, :, 0 : W - 7], t4[:, :, 4 : W - 3])
        rp_a = nrm.tile([C, H, WP], f32)
        nc.vector.tensor_add(rp_a[:], t8[:, :, 0:WP], t2[:, :, 8 : 8 + WP])
        rp = nrm.tile([C, H, WP], f32)
        nc.vector.tensor_add(rp[:], rp_a[:], sq[:, :, 10 : 10 + WP])

        u2 = nrm.tile([C, H - 1, WP], f32)
        nc.vector.tensor_add(u2[:], rp[:, 0 : H - 1], rp[:, 1:H])
        u4 = nrm.tile([C, H - 3, WP], f32)
        nc.vector.tensor_add(u4[:], u2[:, 0 : H - 3], u2[:, 2 : H - 1])
        u8 = nrm.tile([C, H - 7, WP], f32)
        nc.vector.tensor_add(u8[:], u4[:, 0 : H - 7], u4[:, 4 : H - 3])
        nc_a = nrm.tile([C, HP, WP], f32)
        nc.vector.tensor_add(nc_a[:], u8[:, 0:HP], u2[:, 8 : 8 + HP])
        normc = nrm.tile([C, HP, WP], f32)
        nc.vector.tensor_add(normc[:], nc_a[:], rp[:, 10 : 10 + HP])

        NL = HALF * WP  # 450: l columns per l-chunk
        ncv = normc[:].rearrange("p y x -> p (y x)")
        normc_16 = nrm.tile([C, 2 * NL], f16)
        nc.vector.tensor_copy(normc_16[:], ncv)

        # ---- main correlation matmuls.  121 shifts = 60 packed pairs + 1
        # K=64 single (dy=10, dx=10).
        row_chunks = [(0, 4), (4, 4), (8, 4), (12, 3)]

        fp8_cells = set(FP8_CELLS)

        def emit_chunk_mms(r0, nr, j_order=(0, 1), fp8_late=False):
            M = nr * WP

            def emit_bf16(j):
                first = True
                for bi, dx in enumerate(dx_bases):
                    dys = range(KER) if dx < 10 else range(0, KER, 2)
                    for dy in dys:
                        if dx < 10 and ((bi // 2) * 2, dy) in fp8_cells:
                            continue  # covered by a DoubleRow cell below
                        kp = C if (dx == 10 and dy == 10) else 2 * C
                        lhsT = im2c[bi][0:kp, r0 + dy : r0 + dy + nr, :]
                        rhs = im1c[bi][0:kp, HALF * j + dy : HALF * j + dy + HALF, :]
                        last = dx == 10 and dy == 10 and not FP8_CELLS
                        nc.tensor.matmul(ps[j][0:M], lhsT, rhs, start=first, stop=last)
                        first = False

            def emit_fp8(j):
                for ci, (ap, dy) in enumerate(FP8_CELLS):
                    last = ci == len(FP8_CELLS) - 1
                    lhsT = im2c8[
                        :, ap : ap + 2, WP * (r0 + dy) : WP * (r0 + dy) + M
                    ]
                    rhs = im1c8[
                        :, ap : ap + 2, HALF * j + dy : HALF * j + dy + HALF, :
                    ]
                    nc.tensor.matmul(
                        ps[j][0:M], lhsT, rhs, start=False, stop=last, perf_mode=DR
                    )

            ps = [
                psum.tile([128, NL], f32, tag="ps", name=f"ps_{r0}_{j}")
                for j in range(2)
            ]
            if fp8_late:
                # first chunk: alternate j0/j1 per bi pair so each fresh
                # cast buys twice the PE runway (the DVE produces a
                # compacted pair every ~1.7us but one j consumes it in
                # ~1us), and all DoubleRow cells go last so the fp8
                # casts are long done.
                first = [True, True]
                for bi, dx in enumerate(dx_bases):
                    dys = range(KER) if dx < 10 else range(0, KER, 2)
                    for j in j_order:
                        for dy in dys:
                            if dx < 10 and ((bi // 2) * 2, dy) in fp8_cells:
                                continue
                            kp = C if (dx == 10 and dy == 10) else 2 * C
                            lhsT = im2c[bi][0:kp, r0 + dy : r0 + dy + nr, :]
                            rhs = im1c[bi][
                                0:kp, HALF * j + dy : HALF * j + dy + HALF, :
                            ]
                            nc.tensor.matmul(
                                ps[j][0:M], lhsT, rhs, start=first[j], stop=False
                            )
                            first[j] = False
                for j in j_order:
                    emit_fp8(j)
            else:
                for j in j_order:
                    emit_bf16(j)
                    emit_fp8(j)
            return ps

        red_all = reds.tile([128, 4], f32, name="red_all")
        nc.vector.memset(red_all[:], 0.0)

        def emit_epilogue(ci, r0, nr, ps, j_order=(0, 1)):
            # fused (psum * inv) -> running max: second j chains off the
            # first via the reduce's initial-value AP, writing the final
            # column of red_all directly.
            M = nr * WP
            ja, jb = j_order
            USE_TTR = False
            if USE_TTR:
                reda = reds.tile([128, 1], f32, tag="red", name=f"red_{r0}")
                for j, acc, init in (
                    (ja, reda, -3.0e38),
                    (jb, red_all, reda),
                ):
                    sc = scr.tile([128, NL], f32, tag="sc", name=f"sc{j}_{r0}")
                    acc_ap = (
                        acc[0:M, ci : ci + 1] if acc is red_all else acc[0:M]
                    )
                    nc.vector.tensor_tensor_reduce(
                        out=sc[0:M],
                        in0=ps[j][0:M],
                        in1=inv_bc[0:M, NL * j : NL * (j + 1)],
                        scale=1.0,
                        scalar=init if isinstance(init, float) else init[0:M],
                        op0=MULT,
                        op1=MAX,
                        accum_out=acc_ap,
                    )
                return
            red = [None, None]
            for j in j_order:
                sc = scr.tile([128, NL], f32, tag="sc", name=f"sc{j}_{r0}")
                red[j] = reds.tile([128, 1], f32, tag="red", name=f"red{j}_{r0}")
                nc.vector.tensor_tensor(
                    out=sc[0:M],
                    in0=ps[j][0:M],
                    in1=inv_bc[0:M, NL * j : NL * (j + 1)],
                    op=MULT,
                )
                nc.vector.tensor_reduce(
                    out=red[j][0:M], in_=sc[0:M], axis=mybir.AxisListType.X, op=MAX
                )
            nc.vector.tensor_tensor(
                out=red_all[0:M, ci : ci + 1], in0=red[0][0:M], in1=red[1][0:M], op=MAX
            )

        chunk_ps = {}
        chunk_ps[0] = emit_chunk_mms(*row_chunks[0], fp8_late=bool(FP8_CELLS))

        # norm matmul group 1: fp16 channel sum -> sqrt (scalar engine
        # writes fp16 directly).  The whole chain runs at fp16 DVE rate;
        # 1/max(norm,1e-4) == min(1/norm, 1e4) folds the eps clamp into
        # one DVE min on the fp16 row.
        norm_16 = nrm.tile([1, 2 * NL], f16)
        inv_16 = nrm.tile([1, 2 * NL], f16)
        for j in range(2):
            nm = psum.tile([1, NL], f32, tag="ps", name=f"nm_{j}")
            sl = slice(NL * j, NL * (j + 1))
            nc.tensor.matmul(nm[:], ones_k[:], normc_16[:, sl], start=True, stop=True)
            nc.scalar.activation(norm_16[:, sl], nm[:], SQRT)

        chunk_ps[1] = emit_chunk_mms(*row_chunks[1])

        with nc.allow_low_precision(reason="1/norm only needs ~1e-3"):
            nc.vector.reciprocal(inv_16[:], norm_16[:])
            nc.vector.tensor_scalar_min(inv_16[:], inv_16[:], 1.0e4)

        inv_bc = nrm.tile([128, 2 * NL], f32)
        for j in range(2):
            ip = psum.tile([128, NL], f32, tag="ps", name=f"ip_{j}")
            sl = slice(NL * j, NL * (j + 1))
            nc.tensor.matmul(ip[:], ones_m[:], inv_16[:, sl], start=True, stop=True)
            nc.vector.tensor_copy(inv_bc[:, sl], ip[:])

        chunk_ps[2] = emit_chunk_mms(*row_chunks[2])
        emit_epilogue(0, *row_chunks[0], chunk_ps[0])
        # last chunk: j=1 matmuls first so the final accumulation group
        # (j=0) closes last and epilogue(j=1) overlaps its stream.
        chunk_ps[3] = emit_chunk_mms(*row_chunks[3], j_order=(1, 0))
        emit_epilogue(1, *row_chunks[1], chunk_ps[1])
        emit_epilogue(2, *row_chunks[2], chunk_ps[2])
        emit_epilogue(3, *row_chunks[3], chunk_ps[3], j_order=(1, 0))
        nc.gpsimd.dma_start(out_d, red_all[:])

    nc.compile()
    return nc


def _get_program():
    global _PROGRAM
    if _PROGRAM is None:
        _PROGRAM = _build_program()
    return _PROGRAM


def make_in_maps(im1: np.ndarray, im2: np.ndarray):
    in_maps = []
    for b in range(B):
        for h in range(2):
            in_maps.append(
                {
                    "im1": np.ascontiguousarray(im1[b], dtype=np.float32),
                    "im2s": np.ascontiguousarray(
                        im2[b][:, HALF * h : HALF * h + IM2_ROWS, :], dtype=np.float32
                    ),
                }
            )
    return in_maps


ROW_CHUNKS = [(0, 4), (4, 4), (8, 4), (12, 3)]


def _half_from_cols(cols):
    half = np.empty((HALF * WP,), dtype=np.float32)
    for ci, (r0, nr) in enumerate(ROW_CHUNKS):
        half[WP * r0 : WP * r0 + nr * WP] = cols[0 : nr * WP, ci]
    return half.reshape(HALF, WP)


def assemble(results):
    out = np.empty((B, 1, HP, WP), dtype=np.float32)
    for b in range(B):
        top = _half_from_cols(results[2 * b]["out"])
        bot = _half_from_cols(results[2 * b + 1]["out"])
        out[b, 0] = np.concatenate([top, bot], axis=0)
    return out


def run(im1: np.ndarray, im2: np.ndarray, trace: bool = False):
    from concourse import bass_utils

    nc = _get_program()
    res = bass_utils.run_bass_kernel_spmd(
        nc, make_in_maps(im1, im2), core_ids=list(range(N_CORES)), trace=trace
    )
    return assemble(res.results), res


def kernel(im1: np.ndarray, im2: np.ndarray) -> np.ndarray:
    out, _ = run(np.asarray(im1), np.asarray(im2))
    return out


# revision 28
# speedup vs baseline: 1.1479x; 1.1479x over previous
"""Trainium2 Bass kernel for nn_AttnBlock: dynamic-filter correlation.

Math (per sample b):
  p1[l, :]  = 11x11x64 patch of im1 at position l (l over 30x30)
  scores[p, l] = <im2 patch at p, p1[l] / max(||p1[l]||, 1e-4)>
  out[p] = max_l scores[p, l]

Decomposition used on device (per core = one (sample, p-half) pair):
  scores_un[p, l] = sum_{dy,dx} sum_c im2[c, p+(dy,dx)] * im1[c, l+(dy,dx)]
computed as 121 shift-matmuls (contraction over channels) accumulated in
PSUM, two shifts packed per matmul (K=128 bf16).  Each image is DMAed
twice: partitions 0..63 hold the raw image and partitions 64..127 hold
it shifted by one element (flat +1), which bakes the (dx, dx+1) pair
shift into the data.  The stationary operand (walrus requires a single
free dim) uses six width-30 dx-compacted copies of im2 built with one
partition-aligned DVE copy each; the dx=10 tiles (which need a +1 ROW
shift in the upper half) are built from the x tiles: cast the lower
half, then a small SBUF->SBUF DMA replicates it one row up into
partitions 64..127.  This halves HBM traffic vs also loading y-shifted
image copies.  Norms: separable 11x11 box sum of im1^2 (shift-add log
tree on DVE), one fp16 ones-matmul per l-half for the channel sum and
one for the rank-1 partition broadcast of 1/norm (fused fp32 matmuls
silently return zeros at M=1/K=1 on TRN2; single fp16 is accurate to
~5e-4 which is subdominant to the bf16 matmul error).  Scale + max-
over-l run on DVE per PSUM tile.

Sharding: 8 cores = 4 samples x 2 halves of the output-row dim (pure
data parallel, no cross-core communication).
"""

import sys

import numpy as np

if "/opt/trn_rl_repo" not in sys.path:
    sys.path.insert(0, "/opt/trn_rl_repo")

B = 4
C = 64
H = W = 40
KER = 11
HP = WP = H - KER + 1  # 30
HALF = HP // 2  # 15 output rows per core
N_CORES = 2 * B
IM2_ROWS = HALF + KER - 1  # 25 input rows needed per half

_PROGRAM = None

# (dx-pair-base, dy) cells computed as a single fp8e4 DoubleRow matmul
# (4 shifts each: dx in {2ap..2ap+3} x {dy}) instead of two bf16 matmuls.
# Each cell cuts one 450-cycle matmul per (chunk, j) but adds fp8
# quantization noise on its 4/121 of the contraction; pure-fp8 measures
# rel_err 2.2e-2 vs the 2e-2 gate, so the count below keeps the noise
# at ~2.2e-2 * sqrt(4*ncells/121) plus bf16's 1.5e-3.
FP8_CELLS = tuple((ap, dy) for ap in (0, 2) for dy in range(6))


def _build_program():
    import concourse.bass as bass
    import concourse.tile as tile
    from concourse import bacc

    mybir = bass.mybir
    dt = mybir.dt
    f32 = dt.float32
    f32r = dt.float32r
    f16 = dt.float16
    bf16 = dt.bfloat16
    from contextlib import ExitStack

    nc = bacc.Bacc(
        "TRN2",
        target_bir_lowering=False,
        debug=False,
        enable_asserts=False,
        num_devices=N_CORES,
    )
    im1_d = nc.dram_tensor("im1", [C, H, W], f32, kind="ExternalInput").ap()
    im2_d = nc.dram_tensor("im2s", [C, IM2_ROWS, W], f32, kind="ExternalInput").ap()
    out_d = nc.dram_tensor("out", [128, 4], f32, kind="ExternalOutput").ap()

    MM_DT = bf16
    f8 = dt.float8e4
    DR = mybir.MatmulPerfMode.DoubleRow
    MULT = mybir.AluOpType.mult
    MAX = mybir.AluOpType.max
    SQUARE = mybir.ActivationFunctionType.Square
    SQRT = mybir.ActivationFunctionType.Sqrt
    COPY = mybir.ActivationFunctionType.Copy

    im1_flat = im1_d.rearrange("c y x -> c (y x)").bitcast(f32r)
    im2_flat = im2_d.rearrange("c y x -> c (y x)").bitcast(f32r)
    N1 = H * W
    N2 = IM2_ROWS * W

    with tile.TileContext(nc) as tc, ExitStack() as ctx:
        consts = ctx.enter_context(tc.tile_pool(name="consts", bufs=1))
        imgs = ctx.enter_context(tc.tile_pool(name="imgs", bufs=1))
        nrm = ctx.enter_context(tc.tile_pool(name="nrm", bufs=1))
        scr = ctx.enter_context(tc.tile_pool(name="scr", bufs=2))
        reds = ctx.enter_context(tc.tile_pool(name="reds", bufs=6))
        psum = ctx.enter_context(tc.tile_pool(name="psum", bufs=8, space="PSUM"))

        # Dual-shift image tiles (contiguous DMAs): partitions 0..63 raw,
        # 64..127 flat-shifted by +1 (x+1).  The wrap columns are never
        # addressed by the operand APs below.  Kicks spread across the
        # sync/scalar/gpsimd queues so descriptors land early.
        im2x = imgs.tile([128, IM2_ROWS, W], f32r)
        im1x = imgs.tile([128, H, W], f32r)
        im1x_up = im1x[C : 2 * C].rearrange("p y x -> p (y x)")
        # im1x upper kicks go first on their engines: that group gates the
        # first matmul's rhs cast and historically finished last.
        nc.sync.dma_start(im1x_up[0 : C // 2, 0 : N1 - 1], im1_flat[0 : C // 2, 1:N1])
        nc.gpsimd.dma_start(
            im1x_up[C // 2 : C, 0 : N1 - 1], im1_flat[C // 2 : C, 1:N1]
        )
        nc.scalar.dma_start(im1x[0:C], im1_flat)
        nc.sync.dma_start(im2x[0:C], im2_flat)
        nc.gpsimd.dma_start(
            im2x[C : 2 * C].rearrange("p y x -> p (y x)")[:, 0 : N2 - 1],
            im2_flat[:, 1:N2],
        )

        ones_k = consts.tile([C, 1], f16)
        nc.vector.memset(ones_k[:], 1.0)
        ones_m = consts.tile([1, 128], f16)
        nc.vector.memset(ones_m[:], 1.0)

        # Width-30 compacted operand tiles in MM_DT: the stationary side
        # must be a single-free-dim AP, and a contiguous moving side
        # streams ~6% faster than strided reads.  One partition-aligned
        # DVE cast per tile for dx<10 (pair shift already baked into the
        # source's upper half).  The dx=10 tiles get their upper (row+1
        # shifted) half from a small SBUF->SBUF DMA of their own lower
        # half, so no y-shifted HBM loads are needed.  bi=0 casts go
        # first so the first matmul's operands are ready ASAP.
        dx_bases = [0, 2, 4, 6, 8, 10]
        im1c = []
        im2c = []
        # fp8 copies of the dx-compacted tiles for a=0..3, one contiguous
        # tile per image so a DoubleRow k-tile pair (a, a+1) is a simple
        # stride-750/1200 free dim.  They sit where the y staging tiles
        # used to, followed by a pad that restores the bf16 tiles' SBUF
        # offsets: the PE streams ~20% slower when the stationary and
        # moving operand tiles land on conflicting subbanks, so keep the
        # known-good absolute placement fixed.
        # the dual-fp8 LDWEIGHTS k-tile stride must be a multiple of 16
        # bytes, so the im2 fp8 tile pads each dx-base block from 750 to
        # 768 bytes (im1's 1200 is already a multiple of 16).
        S2 = 768
        n8 = len({ap for ap, _ in FP8_CELLS} | {ap + 1 for ap, _ in FP8_CELLS})
        if n8:
            im2c8 = imgs.tile([128, n8, S2], f8, name="im2c8")
            im1c8 = imgs.tile([128, n8, H, WP], f8, name="im1c8")
        pad_f32 = (10400 - n8 * (S2 + H * WP)) // 4
        imgs.tile([128, pad_f32], f32, name="pad")
        with tc.high_priority():
            for bi, dx in enumerate(dx_bases):
                c2 = imgs.tile([128, IM2_ROWS, WP], MM_DT, name=f"im2c_{bi}")
                c1 = imgs.tile([128, H, WP], MM_DT, name=f"im1c_{bi}")
                if dx < 10:
                    nc.vector.tensor_copy(c2[:], im2x[:, :, dx : dx + WP].bitcast(f32))
                    if bi < 2:
                        # scalar runs these concurrently with the im2c casts
                        # on DVE so the first matmuls' operands close early
                        nc.scalar.activation(
                            c1[:], im1x[:, :, dx : dx + WP].bitcast(f32), COPY
                        )
                    else:
                        nc.vector.tensor_copy(
                            c1[:], im1x[:, :, dx : dx + WP].bitcast(f32)
                        )
                else:
                    nc.vector.tensor_copy(
                        c2[0:C], im2x[0:C, :, dx : dx + WP].bitcast(f32)
                    )
                    nc.vector.tensor_copy(
                        c1[0:C], im1x[0:C, :, dx : dx + WP].bitcast(f32)
                    )
                    nc.gpsimd.dma_start(
                        c2[C : 2 * C, 0 : IM2_ROWS - 1, :], c2[0:C, 1:IM2_ROWS, :]
                    )
                    nc.gpsimd.dma_start(c1[C : 2 * C, 0 : H - 1, :], c1[0:C, 1:H, :])
                im2c.append(c2)
                im1c.append(c1)
                # fp8 copies trail the bf16 ones they duplicate; they are
                # only consumed after each group's ~37 bf16 matmuls.
                if n8 and bi < n8:
                    nc.vector.tensor_copy(
                        im2c8[:, bi, 0 : IM2_ROWS * WP].rearrange(
                            "p (y x) -> p y x", y=IM2_ROWS
                        ),
                        im2x[:, :, dx : dx + WP].bitcast(f32),
                    )
                    nc.scalar.activation(
                        im1c8[:, bi], im1x[:, :, dx : dx + WP].bitcast(f32), COPY
                    )

        # ---- norm DVE chain: separable 11x11 box sum of im1^2 over (y, x).
        # Shift-add log tree: widths 1->2->4->8->11.
        sq = nrm.tile([C, H, W], f32)
        nc.scalar.activation(sq[:], im1x[0:C].bitcast(f32), SQUARE)

        t2 = nrm.tile([C, H, W - 1], f32)
        nc.vector.tensor_add(t2[:], sq[:, :, 0 : W - 1], sq[:, :, 1:W])
        t4 = nrm.tile([C, H, W - 3], f32)
        nc.vector.tensor_add(t4[:], t2[:, :, 0 : W - 3], t2[:, :, 2 : W - 1])
        t8 = nrm.tile([C, H, W - 7], f32)
        nc.vector.tensor_add(t8[:], t4[:, :, 0 : W - 7], t4[:, :, 4 : W - 3])
        rp_a = nrm.tile([C, H, WP], f32)
        nc.vector.tensor_add(rp_a[:], t8[:, :, 0:WP], t2[:, :, 8 : 8 + WP])
        rp = nrm.tile([C, H, WP], f32)
        nc.vector.tensor_add(rp[:], rp_a[:], sq[:, :, 10 : 10 + WP])

        u2 = nrm.tile([C, H - 1, WP], f32)
        nc.vector.tensor_add(u2[:], rp[:, 0 : H - 1], rp[:, 1:H])
        u4 = nrm.tile([C, H - 3, WP], f32)
        nc.vector.tensor_add(u4[:], u2[:, 0 : H - 3], u2[:, 2 : H - 1])
        u8 = nrm.tile([C, H - 7, WP], f32)
        nc.vector.tensor_add(u8[:], u4[:, 0 : H - 7], u4[:, 4 : H - 3])
        nc_a = nrm.tile([C, HP, WP], f32)
        nc.vector.tensor_add(nc_a[:], u8[:, 0:HP], u2[:, 8 : 8 + HP])
        normc = nrm.tile([C, HP, WP], f32)
        nc.vector.tensor_add(normc[:], nc_a[:], rp[:, 10 : 10 + HP])

        NL = HALF * WP  # 450: l columns per l-chunk
        ncv = normc[:].rearrange("p y x -> p (y x)")
        normc_16 = nrm.tile([C, 2 * NL], f16)
        nc.vector.tensor_copy(normc_16[:], ncv)

        # ---- main correlation matmuls.  121 shifts = 60 packed pairs + 1
        # K=64 single (dy=10, dx=10).
        row_chunks = [(0, 4), (4, 4), (8, 4), (12, 3)]

        fp8_cells = set(FP8_CELLS)

        def emit_chunk_mms(r0, nr, j_order=(0, 1), fp8_late=False):
            M = nr * WP

            def emit_bf16(j):
                first = True
                for bi, dx in enumerate(dx_bases):
                    dys = range(KER) if dx < 10 else range(0, KER, 2)
                    for dy in dys:
                        if dx < 10 and ((bi // 2) * 2, dy) in fp8_cells:
                            continue  # covered by a DoubleRow cell below
                        kp = C if (dx == 10 and dy == 10) else 2 * C
                        lhsT = im2c[bi][0:kp, r0 + dy : r0 + dy + nr, :]
                        rhs = im1c[bi][0:kp, HALF * j + dy : HALF * j + dy + HALF, :]
                        last = dx == 10 and dy == 10 and not FP8_CELLS
                        nc.tensor.matmul(ps[j][0:M], lhsT, rhs, start=first, stop=last)
                        first = False

            def emit_fp8(j):
                for ci, (ap, dy) in enumerate(FP8_CELLS):
                    last = ci == len(FP8_CELLS) - 1
                    lhsT = im2c8[
                        :, ap : ap + 2, WP * (r0 + dy) : WP * (r0 + dy) + M
                    ]
                    rhs = im1c8[
                        :, ap : ap + 2, HALF * j + dy : HALF * j + dy + HALF, :
                    ]
                    nc.tensor.matmul(
                        ps[j][0:M], lhsT, rhs, start=False, stop=last, perf_mode=DR
                    )

            ps = [
                psum.tile([128, NL], f32, tag="ps", name=f"ps_{r0}_{j}")
                for j in range(2)
            ]
            if fp8_late:
                # first chunk: alternate j0/j1 per bi pair so each fresh
                # cast buys twice the PE runway (the DVE produces a
                # compacted pair every ~1.7us but one j consumes it in
                # ~1us), and all DoubleRow cells go last so the fp8
                # casts are long done.
                first = [True, True]
                for bi, dx in enumerate(dx_bases):
                    dys = range(KER) if dx < 10 else range(0, KER, 2)
                    for j in j_order:
                        for dy in dys:
                            if dx < 10 and ((bi // 2) * 2, dy) in fp8_cells:
                                continue
                            kp = C if (dx == 10 and dy == 10) else 2 * C
                            lhsT = im2c[bi][0:kp, r0 + dy : r0 + dy + nr, :]
                            rhs = im1c[bi][
                                0:kp, HALF * j + dy : HALF * j + dy + HALF, :
                            ]
                            nc.tensor.matmul(
                                ps[j][0:M], lhsT, rhs, start=first[j], stop=False
                            )
                            first[j] = False
                for j in j_order:
                    emit_fp8(j)
            else:
                for j in j_order:
                    emit_bf16(j)
                    emit_fp8(j)
            return ps

        red_all = reds.tile([128, 4], f32, name="red_all")
        nc.vector.memset(red_all[:], 0.0)

        def emit_epilogue(ci, r0, nr, ps, j_order=(0, 1)):
            # fused (psum * inv) -> running max: second j chains off the
            # first via the reduce's initial-value AP, writing the final
            # column of red_all directly.
            M = nr * WP
            ja, jb = j_order
            USE_TTR = False
            if USE_TTR:
                reda = reds.tile([128, 1], f32, tag="red", name=f"red_{r0}")
                for j, acc, init in (
                    (ja, reda, -3.0e38),
                    (jb, red_all, reda),
                ):
                    sc = scr.tile([128, NL], f32, tag="sc", name=f"sc{j}_{r0}")
                    acc_ap = (
                        acc[0:M, ci : ci + 1] if acc is red_all else acc[0:M]
                    )
                    nc.vector.tensor_tensor_reduce(
                        out=sc[0:M],
                        in0=ps[j][0:M],
                        in1=inv_bc[0:M, NL * j : NL * (j + 1)],
                        scale=1.0,
                        scalar=init if isinstance(init, float) else init[0:M],
                        op0=MULT,
                        op1=MAX,
                        accum_out=acc_ap,
                    )
                return
            red = [None, None]
            for j in j_order:
                sc = scr.tile([128, NL], f32, tag="sc", name=f"sc{j}_{r0}")
                red[j] = reds.tile([128, 1], f32, tag="red", name=f"red{j}_{r0}")
                nc.vector.tensor_tensor(
                    out=sc[0:M],
                    in0=ps[j][0:M],
                    in1=inv_bc[0:M, NL * j : NL * (j + 1)],
                    op=MULT,
                )
                nc.vector.tensor_reduce(
                    out=red[j][0:M], in_=sc[0:M], axis=mybir.AxisListType.X, op=MAX
                )
            nc.vector.tensor_tensor(
                out=red_all[0:M, ci : ci + 1], in0=red[0][0:M], in1=red[1][0:M], op=MAX
            )

        chunk_ps = {}
        chunk_ps[0] = emit_chunk_mms(*row_chunks[0])
        chunk_ps[1] = emit_chunk_mms(*row_chunks[1])

        # norm matmul group 1: fp16 channel sum -> sqrt (scalar engine
        # writes fp16 directly).  The whole chain runs at fp16 DVE rate;
        # 1/max(norm,1e-4) == min(1/norm, 1e4) folds the eps clamp into
        # one DVE min on the fp16 row.
        norm_16 = nrm.tile([1, 2 * NL], f16)
        inv_16 = nrm.tile([1, 2 * NL], f16)
        for j in range(2):
            nm = psum.tile([1, NL], f32, tag="ps", name=f"nm_{j}")
            sl = slice(NL * j, NL * (j + 1))
            nc.tensor.matmul(nm[:], ones_k[:], normc_16[:, sl], start=True, stop=True)
            nc.scalar.activation(norm_16[:, sl], nm[:], SQRT)

        chunk_ps[2] = emit_chunk_mms(*row_chunks[2])

        with nc.allow_low_precision(reason="1/norm only needs ~1e-3"):
            nc.vector.reciprocal(inv_16[:], norm_16[:])
            nc.vector.tensor_scalar_min(inv_16[:], inv_16[:], 1.0e4)

        inv_bc = nrm.tile([128, 2 * NL], f32)
        for j in range(2):
            ip = psum.tile([128, NL], f32, tag="ps", name=f"ip_{j}")
            sl = slice(NL * j, NL * (j + 1))
            nc.tensor.matmul(ip[:], ones_m[:], inv_16[:, sl], start=True, stop=True)
            nc.vector.tensor_copy(inv_bc[:, sl], ip[:])

        emit_epilogue(0, *row_chunks[0], chunk_ps[0])
        # last chunk: j=1 matmuls first so the final accumulation group
        # (j=0) closes last and epilogue(j=1) overlaps its stream.
        chunk_ps[3] = emit_chunk_mms(*row_chunks[3], j_order=(1, 0))
        emit_epilogue(1, *row_chunks[1], chunk_ps[1])
        emit_epilogue(2, *row_chunks[2], chunk_ps[2])
        emit_epilogue(3, *row_chunks[3], chunk_ps[3], j_order=(1, 0))
        nc.gpsimd.dma_start(out_d, red_all[:])

    nc.compile()
    return nc


def _get_program():
    global _PROGRAM
    if _PROGRAM is None:
        _PROGRAM = _build_program()
    return _PROGRAM


def make_in_maps(im1: np.ndarray, im2: np.ndarray):
    in_maps = []
    for b in range(B):
        for h in range(2):
            in_maps.append(
                {
                    "im1": np.ascontiguousarray(im1[b], dtype=np.float32),
                    "im2s": np.ascontiguousarray(
                        im2[b][:, HALF * h : HALF * h + IM2_ROWS, :], dtype=np.float32
                    ),
                }
            )
    return in_maps


ROW_CHUNKS = [(0, 4), (4, 4), (8, 4), (12, 3)]


def _half_from_cols(cols):
    half = np.empty((HALF * WP,), dtype=np.float32)
    for ci, (r0, nr) in enumerate(ROW_CHUNKS):
        half[WP * r0 : WP * r0 + nr * WP] = cols[0 : nr * WP, ci]
    return half.reshape(HALF, WP)


def assemble(results):
    out = np.empty((B, 1, HP, WP), dtype=np.float32)
    for b in range(B):
        top = _half_from_cols(results[2 * b]["out"])
        bot = _half_from_cols(results[2 * b + 1]["out"])
        out[b, 0] = np.concatenate([top, bot], axis=0)
    return out


def run(im1: np.ndarray, im2: np.ndarray, trace: bool = False):
    from concourse import bass_utils

    nc = _get_program()
    res = bass_utils.run_bass_kernel_spmd(
        nc, make_in_maps(im1, im2), core_ids=list(range(N_CORES)), trace=trace
    )
    return assemble(res.results), res


def kernel(im1: np.ndarray, im2: np.ndarray) -> np.ndarray:
    out, _ = run(np.asarray(im1), np.asarray(im2))
    return out


# revision 29
# speedup vs baseline: 1.1863x; 1.0335x over previous
"""Trainium2 Bass kernel for nn_AttnBlock: dynamic-filter correlation.

Math (per sample b):
  p1[l, :]  = 11x11x64 patch of im1 at position l (l over 30x30)
  scores[p, l] = <im2 patch at p, p1[l] / max(||p1[l]||, 1e-4)>
  out[p] = max_l scores[p, l]

Decomposition used on device (per core = one (sample, p-half) pair):
  scores_un[p, l] = sum_{dy,dx} sum_c im2[c, p+(dy,dx)] * im1[c, l+(dy,dx)]
computed as 121 shift-matmuls (contraction over channels) accumulated in
PSUM, two shifts packed per matmul (K=128 bf16).  Each image is DMAed
twice: partitions 0..63 hold the raw image and partitions 64..127 hold
it shifted by one element (flat +1), which bakes the (dx, dx+1) pair
shift into the data.  The stationary operand (walrus requires a single
free dim) uses six width-30 dx-compacted copies of im2 built with one
partition-aligned DVE copy each; the dx=10 tiles (which need a +1 ROW
shift in the upper half) are built from the x tiles: cast the lower
half, then a small SBUF->SBUF DMA replicates it one row up into
partitions 64..127.  This halves HBM traffic vs also loading y-shifted
image copies.  Norms: separable 11x11 box sum of im1^2 (shift-add log
tree on DVE), one fp16 ones-matmul per l-half for the channel sum and
one for the rank-1 partition broadcast of 1/norm (fused fp32 matmuls
silently return zeros at M=1/K=1 on TRN2; single fp16 is accurate to
~5e-4 which is subdominant to the bf16 matmul error).  Scale + max-
over-l run on DVE per PSUM tile.

Sharding: 8 cores = 4 samples x 2 halves of the output-row dim (pure
data parallel, no cross-core communication).
"""

import sys

import numpy as np

if "/opt/trn_rl_repo" not in sys.path:
    sys.path.insert(0, "/opt/trn_rl_repo")

B = 4
C = 64
H = W = 40
KER = 11
HP = WP = H - KER + 1  # 30
HALF = HP // 2  # 15 output rows per core
N_CORES = 2 * B
IM2_ROWS = HALF + KER - 1  # 25 input rows needed per half

_PROGRAM = None

# (dx-pair-base, dy) cells computed as a single fp8e4 DoubleRow matmul
# (4 shifts each: dx in {2ap..2ap+3} x {dy}) instead of two bf16 matmuls.
# Each cell cuts one 450-cycle matmul per (chunk, j) but adds fp8
# quantization noise on its 4/121 of the contraction; pure-fp8 measures
# rel_err 2.2e-2 vs the 2e-2 gate, so the count below keeps the noise
# at ~2.2e-2 * sqrt(4*ncells/121) plus bf16's 1.5e-3.
FP8_CELLS = tuple((ap, dy) for ap in (0, 2) for dy in range(6))


def _build_program():
    import concourse.bass as bass
    import concourse.tile as tile
    from concourse import bacc

    mybir = bass.mybir
    dt = mybir.dt
    f32 = dt.float32
    f32r = dt.float32r
    f16 = dt.float16
    bf16 = dt.bfloat16
    from contextlib import ExitStack

    nc = bacc.Bacc(
        "TRN2",
        target_bir_lowering=False,
        debug=False,
        enable_asserts=False,
        num_devices=N_CORES,
    )
    im1_d = nc.dram_tensor("im1", [C, H, W], f32, kind="ExternalInput").ap()
    im2_d = nc.dram_tensor("im2s", [C, IM2_ROWS, W], f32, kind="ExternalInput").ap()
    out_d = nc.dram_tensor("out", [128, 4], f32, kind="ExternalOutput").ap()

    MM_DT = bf16
    f8 = dt.float8e4
    DR = mybir.MatmulPerfMode.DoubleRow
    MULT = mybir.AluOpType.mult
    MAX = mybir.AluOpType.max
    SQUARE = mybir.ActivationFunctionType.Square
    SQRT = mybir.ActivationFunctionType.Sqrt
    COPY = mybir.ActivationFunctionType.Copy

    im1_flat = im1_d.rearrange("c y x -> c (y x)").bitcast(f32r)
    im2_flat = im2_d.rearrange("c y x -> c (y x)").bitcast(f32r)
    N1 = H * W
    N2 = IM2_ROWS * W

    with tile.TileContext(nc) as tc, ExitStack() as ctx:
        consts = ctx.enter_context(tc.tile_pool(name="consts", bufs=1))
        imgs = ctx.enter_context(tc.tile_pool(name="imgs", bufs=1))
        nrm = ctx.enter_context(tc.tile_pool(name="nrm", bufs=1))
        scr = ctx.enter_context(tc.tile_pool(name="scr", bufs=2))
        reds = ctx.enter_context(tc.tile_pool(name="reds", bufs=6))
        psum = ctx.enter_context(tc.tile_pool(name="psum", bufs=8, space="PSUM"))

        # Dual-shift image tiles (contiguous DMAs): partitions 0..63 raw,
        # 64..127 flat-shifted by +1 (x+1).  The wrap columns are never
        # addressed by the operand APs below.  Kicks spread across the
        # sync/scalar/gpsimd queues so descriptors land early.
        im2x = imgs.tile([128, IM2_ROWS, W], f32r)
        im1x = imgs.tile([128, H, W], f32r)
        im1x_up = im1x[C : 2 * C].rearrange("p y x -> p (y x)")
        # im1x upper kicks go first on their engines: that group gates the
        # first matmul's rhs cast and historically finished last.
        nc.sync.dma_start(im1x_up[0 : C // 2, 0 : N1 - 1], im1_flat[0 : C // 2, 1:N1])
        nc.gpsimd.dma_start(
            im1x_up[C // 2 : C, 0 : N1 - 1], im1_flat[C // 2 : C, 1:N1]
        )
        nc.scalar.dma_start(im1x[0:C], im1_flat)
        nc.sync.dma_start(im2x[0:C], im2_flat)
        nc.gpsimd.dma_start(
            im2x[C : 2 * C].rearrange("p y x -> p (y x)")[:, 0 : N2 - 1],
            im2_flat[:, 1:N2],
        )

        ones_k = consts.tile([C, 1], f16)
        nc.vector.memset(ones_k[:], 1.0)
        ones_m = consts.tile([1, 128], f16)
        nc.vector.memset(ones_m[:], 1.0)

        # Width-30 compacted operand tiles in MM_DT: the stationary side
        # must be a single-free-dim AP, and a contiguous moving side
        # streams ~6% faster than strided reads.  One partition-aligned
        # DVE cast per tile for dx<10 (pair shift already baked into the
        # source's upper half).  The dx=10 tiles get their upper (row+1
        # shifted) half from a small SBUF->SBUF DMA of their own lower
        # half, so no y-shifted HBM loads are needed.  bi=0 casts go
        # first so the first matmul's operands are ready ASAP.
        dx_bases = [0, 2, 4, 6, 8, 10]
        im1c = []
        im2c = []
        # fp8 copies of the dx-compacted tiles for a=0..3, one contiguous
        # tile per image so a DoubleRow k-tile pair (a, a+1) is a simple
        # stride-750/1200 free dim.  They sit where the y staging tiles
        # used to, followed by a pad that restores the bf16 tiles' SBUF
        # offsets: the PE streams ~20% slower when the stationary and
        # moving operand tiles land on conflicting subbanks, so keep the
        # known-good absolute placement fixed.
        # the dual-fp8 LDWEIGHTS k-tile stride must be a multiple of 16
        # bytes, so the im2 fp8 tile pads each dx-base block from 750 to
        # 768 bytes (im1's 1200 is already a multiple of 16).
        S2 = 768
        n8 = len({ap for ap, _ in FP8_CELLS} | {ap + 1 for ap, _ in FP8_CELLS})
        if n8:
            im2c8 = imgs.tile([128, n8, S2], f8, name="im2c8")
            im1c8 = imgs.tile([128, n8, H, WP], f8, name="im1c8")
        pad_f32 = (10400 - n8 * (S2 + H * WP)) // 4
        imgs.tile([128, pad_f32], f32, name="pad")
        with tc.high_priority():
            for bi, dx in enumerate(dx_bases):
                c2 = imgs.tile([128, IM2_ROWS, WP], MM_DT, name=f"im2c_{bi}")
                c1 = imgs.tile([128, H, WP], MM_DT, name=f"im1c_{bi}")
                if dx < 10:
                    nc.vector.tensor_copy(c2[:], im2x[:, :, dx : dx + WP].bitcast(f32))
                    if bi < 2:
                        # scalar runs these concurrently with the im2c casts
                        # on DVE so the first matmuls' operands close early
                        nc.scalar.activation(
                            c1[:], im1x[:, :, dx : dx + WP].bitcast(f32), COPY
                        )
                    else:
                        nc.vector.tensor_copy(
                            c1[:], im1x[:, :, dx : dx + WP].bitcast(f32)
                        )
                else:
                    nc.vector.tensor_copy(
                        c2[0:C], im2x[0:C, :, dx : dx + WP].bitcast(f32)
                    )
                    nc.vector.tensor_copy(
                        c1[0:C], im1x[0:C, :, dx : dx + WP].bitcast(f32)
                    )
                    nc.gpsimd.dma_start(
                        c2[C : 2 * C, 0 : IM2_ROWS - 1, :], c2[0:C, 1:IM2_ROWS, :]
                    )
                    nc.gpsimd.dma_start(c1[C : 2 * C, 0 : H - 1, :], c1[0:C, 1:H, :])
                im2c.append(c2)
                im1c.append(c1)
                # fp8 copies trail the bf16 ones they duplicate; they are
                # only consumed after each group's ~37 bf16 matmuls.
                if n8 and bi < n8:
                    nc.vector.tensor_copy(
                        im2c8[:, bi, 0 : IM2_ROWS * WP].rearrange(
                            "p (y x) -> p y x", y=IM2_ROWS
                        ),
                        im2x[:, :, dx : dx + WP].bitcast(f32),
                    )
                    nc.scalar.activation(
                        im1c8[:, bi], im1x[:, :, dx : dx + WP].bitcast(f32), COPY
                    )

        # ---- norm DVE chain: separable 11x11 box sum of im1^2 over (y, x).
        # Shift-add log tree: widths 1->2->4->8->11.
        sq = nrm.tile([C, H, W], f32)
        nc.scalar.activation(sq[:], im1x[0:C].bitcast(f32), SQUARE)

        t2 = nrm.tile([C, H, W - 1], f32)
        nc.vector.tensor_add(t2[:], sq[:, :, 0 : W - 1], sq[:, :, 1:W])
        t4 = nrm.tile([C, H, W - 3], f32)
        nc.vector.tensor_add(t4[:], t2[:, :, 0 : W - 3], t2[:, :, 2 : W - 1])
        t8 = nrm.tile([C, H, W - 7], f32)
        nc.vector.tensor_add(t8[:], t4[:, :, 0 : W - 7], t4[:, :, 4 : W - 3])
        rp_a = nrm.tile([C, H, WP], f32)
        nc.vector.tensor_add(rp_a[:], t8[:, :, 0:WP], t2[:, :, 8 : 8 + WP])
        rp = nrm.tile([C, H, WP], f32)
        nc.vector.tensor_add(rp[:], rp_a[:], sq[:, :, 10 : 10 + WP])

        u2 = nrm.tile([C, H - 1, WP], f32)
        nc.vector.tensor_add(u2[:], rp[:, 0 : H - 1], rp[:, 1:H])
        u4 = nrm.tile([C, H - 3, WP], f32)
        nc.vector.tensor_add(u4[:], u2[:, 0 : H - 3], u2[:, 2 : H - 1])
        u8 = nrm.tile([C, H - 7, WP], f32)
        nc.vector.tensor_add(u8[:], u4[:, 0 : H - 7], u4[:, 4 : H - 3])
        nc_a = nrm.tile([C, HP, WP], f32)
        nc.vector.tensor_add(nc_a[:], u8[:, 0:HP], u2[:, 8 : 8 + HP])
        normc = nrm.tile([C, HP, WP], f32)
        nc.vector.tensor_add(normc[:], nc_a[:], rp[:, 10 : 10 + HP])

        NL = HALF * WP  # 450: l columns per l-chunk
        ncv = normc[:].rearrange("p y x -> p (y x)")
        normc_16 = nrm.tile([C, 2 * NL], f16)
        nc.vector.tensor_copy(normc_16[:], ncv)

        # ---- main correlation matmuls.  121 shifts = 60 packed pairs + 1
        # K=64 single (dy=10, dx=10).
        row_chunks = [(0, 4), (4, 4), (8, 4), (12, 3)]

        fp8_cells = set(FP8_CELLS)

        def emit_chunk_mms(r0, nr, j_order=(0, 1), fp8_late=False):
            M = nr * WP

            def emit_bf16(j):
                first = True
                for bi, dx in enumerate(dx_bases):
                    dys = range(KER) if dx < 10 else range(0, KER, 2)
                    for dy in dys:
                        if dx < 10 and ((bi // 2) * 2, dy) in fp8_cells:
                            continue  # covered by a DoubleRow cell below
                        kp = C if (dx == 10 and dy == 10) else 2 * C
                        lhsT = im2c[bi][0:kp, r0 + dy : r0 + dy + nr, :]
                        rhs = im1c[bi][0:kp, HALF * j + dy : HALF * j + dy + HALF, :]
                        last = dx == 10 and dy == 10 and not FP8_CELLS
                        nc.tensor.matmul(ps[j][0:M], lhsT, rhs, start=first, stop=last)
                        first = False

            def emit_fp8(j):
                for ci, (ap, dy) in enumerate(FP8_CELLS):
                    last = ci == len(FP8_CELLS) - 1
                    lhsT = im2c8[
                        :, ap : ap + 2, WP * (r0 + dy) : WP * (r0 + dy) + M
                    ]
                    rhs = im1c8[
                        :, ap : ap + 2, HALF * j + dy : HALF * j + dy + HALF, :
                    ]
                    nc.tensor.matmul(
                        ps[j][0:M], lhsT, rhs, start=False, stop=last, perf_mode=DR
                    )

            ps = [
                psum.tile([128, NL], f32, tag="ps", name=f"ps_{r0}_{j}")
                for j in range(2)
            ]
            if fp8_late:
                # first chunk: alternate j0/j1 per bi pair so each fresh
                # cast buys twice the PE runway (the DVE produces a
                # compacted pair every ~1.7us but one j consumes it in
                # ~1us), and all DoubleRow cells go last so the fp8
                # casts are long done.
                first = [True, True]
                for bi, dx in enumerate(dx_bases):
                    dys = range(KER) if dx < 10 else range(0, KER, 2)
                    for j in j_order:
                        for dy in dys:
                            if dx < 10 and ((bi // 2) * 2, dy) in fp8_cells:
                                continue
                            kp = C if (dx == 10 and dy == 10) else 2 * C
                            lhsT = im2c[bi][0:kp, r0 + dy : r0 + dy + nr, :]
                            rhs = im1c[bi][
                                0:kp, HALF * j + dy : HALF * j + dy + HALF, :
                            ]
                            nc.tensor.matmul(
                                ps[j][0:M], lhsT, rhs, start=first[j], stop=False
                            )
                            first[j] = False
                for j in j_order:
                    emit_fp8(j)
            else:
                for j in j_order:
                    emit_bf16(j)
                    emit_fp8(j)
            return ps

        red_all = reds.tile([128, 4], f32, name="red_all")
        nc.vector.memset(red_all[:], 0.0)

        def emit_epilogue(ci, r0, nr, ps, j_order=(0, 1)):
            # fused (psum * inv) -> running max: second j chains off the
            # first via the reduce's initial-value AP, writing the final
            # column of red_all directly.
            M = nr * WP
            ja, jb = j_order
            USE_TTR = False
            if USE_TTR:
                reda = reds.tile([128, 1], f32, tag="red", name=f"red_{r0}")
                for j, acc, init in (
                    (ja, reda, -3.0e38),
                    (jb, red_all, reda),
                ):
                    sc = scr.tile([128, NL], f32, tag="sc", name=f"sc{j}_{r0}")
                    acc_ap = (
                        acc[0:M, ci : ci + 1] if acc is red_all else acc[0:M]
                    )
                    nc.vector.tensor_tensor_reduce(
                        out=sc[0:M],
                        in0=ps[j][0:M],
                        in1=inv_bc[0:M, NL * j : NL * (j + 1)],
                        scale=1.0,
                        scalar=init if isinstance(init, float) else init[0:M],
                        op0=MULT,
                        op1=MAX,
                        accum_out=acc_ap,
                    )
                return
            red = [None, None]
            for j in j_order:
                sc = scr.tile([128, NL], f32, tag="sc", name=f"sc{j}_{r0}")
                red[j] = reds.tile([128, 1], f32, tag="red", name=f"red{j}_{r0}")
                nc.vector.tensor_tensor(
                    out=sc[0:M],
                    in0=ps[j][0:M],
                    in1=inv_bc[0:M, NL * j : NL * (j + 1)],
                    op=MULT,
                )
                nc.vector.tensor_reduce(
                    out=red[j][0:M], in_=sc[0:M], axis=mybir.AxisListType.X, op=MAX
                )
            nc.vector.tensor_tensor(
                out=red_all[0:M, ci : ci + 1], in0=red[0][0:M], in1=red[1][0:M], op=MAX
            )

        chunk_ps = {}
        chunk_ps[0] = emit_chunk_mms(*row_chunks[0], fp8_late=bool(FP8_CELLS))
        chunk_ps[1] = emit_chunk_mms(*row_chunks[1])

        # norm matmul group 1: fp16 channel sum -> sqrt (scalar engine
        # writes fp16 directly).  The whole chain runs at fp16 DVE rate;
        # 1/max(norm,1e-4) == min(1/norm, 1e4) folds the eps clamp into
        # one DVE min on the fp16 row.
        norm_16 = nrm.tile([1, 2 * NL], f16)
        inv_16 = nrm.tile([1, 2 * NL], f16)
        for j in range(2):
            nm = psum.tile([1, NL], f32, tag="ps", name=f"nm_{j}")
            sl = slice(NL * j, NL * (j + 1))
            nc.tensor.matmul(nm[:], ones_k[:], normc_16[:, sl], start=True, stop=True)
            nc.scalar.activation(norm_16[:, sl], nm[:], SQRT)

        chunk_ps[2] = emit_chunk_mms(*row_chunks[2])

        with nc.allow_low_precision(reason="1/norm only needs ~1e-3"):
            nc.vector.reciprocal(inv_16[:], norm_16[:])
            nc.vector.tensor_scalar_min(inv_16[:], inv_16[:], 1.0e4)

        inv_bc = nrm.tile([128, 2 * NL], f32)
        for j in range(2):
            ip = psum.tile([128, NL], f32, tag="ps", name=f"ip_{j}")
            sl = slice(NL * j, NL * (j + 1))
            nc.tensor.matmul(ip[:], ones_m[:], inv_16[:, sl], start=True, stop=True)
            nc.vector.tensor_copy(inv_bc[:, sl], ip[:])

        emit_epilogue(0, *row_chunks[0], chunk_ps[0])
        # last chunk: j=1 matmuls first so the final accumulation group
        # (j=0) closes last and epilogue(j=1) overlaps its stream.
        chunk_ps[3] = emit_chunk_mms(*row_chunks[3], j_order=(1, 0))
        emit_epilogue(1, *row_chunks[1], chunk_ps[1])
        emit_epilogue(2, *row_chunks[2], chunk_ps[2])
        emit_epilogue(3, *row_chunks[3], chunk_ps[3], j_order=(1, 0))
        nc.gpsimd.dma_start(out_d, red_all[:])

    nc.compile()
    return nc


def _get_program():
    global _PROGRAM
    if _PROGRAM is None:
        _PROGRAM = _build_program()
    return _PROGRAM


def make_in_maps(im1: np.ndarray, im2: np.ndarray):
    in_maps = []
    for b in range(B):
        for h in range(2):
            in_maps.append(
                {
                    "im1": np.ascontiguousarray(im1[b], dtype=np.float32),
                    "im2s": np.ascontiguousarray(
                        im2[b][:, HALF * h : HALF * h + IM2_ROWS, :], dtype=np.float32
                    ),
                }
            )
    return in_maps


ROW_CHUNKS = [(0, 4), (4, 4), (8, 4), (12, 3)]


def _half_from_cols(cols):
    half = np.empty((HALF * WP,), dtype=np.float32)
    for ci, (r0, nr) in enumerate(ROW_CHUNKS):
        half[WP * r0 : WP * r0 + nr * WP] = cols[0 : nr * WP, ci]
    return half.reshape(HALF, WP)


def assemble(results):
    out = np.empty((B, 1, HP, WP), dtype=np.float32)
    for b in range(B):
        top = _half_from_cols(results[2 * b]["out"])
        bot = _half_from_cols(results[2 * b + 1]["out"])
        out[b, 0] = np.concatenate([top, bot], axis=0)
    return out


def run(im1: np.ndarray, im2: np.ndarray, trace: bool = False):
    from concourse import bass_utils

    nc = _get_program()
    res = bass_utils.run_bass_kernel_spmd(
        nc, make_in_maps(im1, im2), core_ids=list(range(N_CORES)), trace=trace
    )
    return assemble(res.results), res


def kernel(im1: np.ndarray, im2: np.ndarray) -> np.ndarray:
    out, _ = run(np.asarray(im1), np.asarray(im2))
    return out


# revision 30
# speedup vs baseline: 1.1959x; 1.0081x over previous
"""Trainium2 Bass kernel for nn_AttnBlock: dynamic-filter correlation.

Math (per sample b):
  p1[l, :]  = 11x11x64 patch of im1 at position l (l over 30x30)
  scores[p, l] = <im2 patch at p, p1[l] / max(||p1[l]||, 1e-4)>
  out[p] = max_l scores[p, l]

Decomposition used on device (per core = one (sample, p-half) pair):
  scores_un[p, l] = sum_{dy,dx} sum_c im2[c, p+(dy,dx)] * im1[c, l+(dy,dx)]
computed as 121 shift-matmuls (contraction over channels) accumulated in
PSUM, two shifts packed per matmul (K=128 bf16).  Each image is DMAed
twice: partitions 0..63 hold the raw image and partitions 64..127 hold
it shifted by one element (flat +1), which bakes the (dx, dx+1) pair
shift into the data.  The stationary operand (walrus requires a single
free dim) uses six width-30 dx-compacted copies of im2 built with one
partition-aligned DVE copy each; the dx=10 tiles (which need a +1 ROW
shift in the upper half) are built from the x tiles: cast the lower
half, then a small SBUF->SBUF DMA replicates it one row up into
partitions 64..127.  This halves HBM traffic vs also loading y-shifted
image copies.  Norms: separable 11x11 box sum of im1^2 (shift-add log
tree on DVE), one fp16 ones-matmul per l-half for the channel sum and
one for the rank-1 partition broadcast of 1/norm (fused fp32 matmuls
silently return zeros at M=1/K=1 on TRN2; single fp16 is accurate to
~5e-4 which is subdominant to the bf16 matmul error).  Scale + max-
over-l run on DVE per PSUM tile.

Sharding: 8 cores = 4 samples x 2 halves of the output-row dim (pure
data parallel, no cross-core communication).
"""

import sys

import numpy as np

if "/opt/trn_rl_repo" not in sys.path:
    sys.path.insert(0, "/opt/trn_rl_repo")

B = 4
C = 64
H = W = 40
KER = 11
HP = WP = H - KER + 1  # 30
HALF = HP // 2  # 15 output rows per core
N_CORES = 2 * B
IM2_ROWS = HALF + KER - 1  # 25 input rows needed per half

_PROGRAM = None

# (dx-pair-base, dy) cells computed as a single fp8e4 DoubleRow matmul
# (4 shifts each: dx in {2ap..2ap+3} x {dy}) instead of two bf16 matmuls.
# Each cell cuts one 450-cycle matmul per (chunk, j) but adds fp8
# quantization noise on its 4/121 of the contraction; pure-fp8 measures
# rel_err 2.2e-2 vs the 2e-2 gate, so the count below keeps the noise
# at ~2.2e-2 * sqrt(4*ncells/121) plus bf16's 1.5e-3.
FP8_CELLS = tuple((ap, dy) for ap in (0, 2) for dy in range(7))


def _build_program():
    import concourse.bass as bass
    import concourse.tile as tile
    from concourse import bacc

    mybir = bass.mybir
    dt = mybir.dt
    f32 = dt.float32
    f32r = dt.float32r
    f16 = dt.float16
    bf16 = dt.bfloat16
    from contextlib import ExitStack

    nc = bacc.Bacc(
        "TRN2",
        target_bir_lowering=False,
        debug=False,
        enable_asserts=False,
        num_devices=N_CORES,
    )
    im1_d = nc.dram_tensor("im1", [C, H, W], f32, kind="ExternalInput").ap()
    im2_d = nc.dram_tensor("im2s", [C, IM2_ROWS, W], f32, kind="ExternalInput").ap()
    out_d = nc.dram_tensor("out", [128, 4], f32, kind="ExternalOutput").ap()

    MM_DT = bf16
    f8 = dt.float8e4
    DR = mybir.MatmulPerfMode.DoubleRow
    MULT = mybir.AluOpType.mult
    MAX = mybir.AluOpType.max
    SQUARE = mybir.ActivationFunctionType.Square
    SQRT = mybir.ActivationFunctionType.Sqrt
    COPY = mybir.ActivationFunctionType.Copy

    im1_flat = im1_d.rearrange("c y x -> c (y x)").bitcast(f32r)
    im2_flat = im2_d.rearrange("c y x -> c (y x)").bitcast(f32r)
    N1 = H * W
    N2 = IM2_ROWS * W

    with tile.TileContext(nc) as tc, ExitStack() as ctx:
        consts = ctx.enter_context(tc.tile_pool(name="consts", bufs=1))
        imgs = ctx.enter_context(tc.tile_pool(name="imgs", bufs=1))
        nrm = ctx.enter_context(tc.tile_pool(name="nrm", bufs=1))
        scr = ctx.enter_context(tc.tile_pool(name="scr", bufs=2))
        reds = ctx.enter_context(tc.tile_pool(name="reds", bufs=6))
        psum = ctx.enter_context(tc.tile_pool(name="psum", bufs=8, space="PSUM"))

        # Dual-shift image tiles (contiguous DMAs): partitions 0..63 raw,
        # 64..127 flat-shifted by +1 (x+1).  The wrap columns are never
        # addressed by the operand APs below.  Kicks spread across the
        # sync/scalar/gpsimd queues so descriptors land early.
        im2x = imgs.tile([128, IM2_ROWS, W], f32r)
        im1x = imgs.tile([128, H, W], f32r)
        im1x_up = im1x[C : 2 * C].rearrange("p y x -> p (y x)")
        # im1x upper kicks go first on their engines: that group gates the
        # first matmul's rhs cast and historically finished last.
        nc.sync.dma_start(im1x_up[0 : C // 2, 0 : N1 - 1], im1_flat[0 : C // 2, 1:N1])
        nc.gpsimd.dma_start(
            im1x_up[C // 2 : C, 0 : N1 - 1], im1_flat[C // 2 : C, 1:N1]
        )
        nc.scalar.dma_start(im1x[0:C], im1_flat)
        nc.sync.dma_start(im2x[0:C], im2_flat)
        nc.gpsimd.dma_start(
            im2x[C : 2 * C].rearrange("p y x -> p (y x)")[:, 0 : N2 - 1],
            im2_flat[:, 1:N2],
        )

        ones_k = consts.tile([C, 1], f16)
        nc.vector.memset(ones_k[:], 1.0)
        ones_m = consts.tile([1, 128], f16)
        nc.vector.memset(ones_m[:], 1.0)

        # Width-30 compacted operand tiles in MM_DT: the stationary side
        # must be a single-free-dim AP, and a contiguous moving side
        # streams ~6% faster than strided reads.  One partition-aligned
        # DVE cast per tile for dx<10 (pair shift already baked into the
        # source's upper half).  The dx=10 tiles get their upper (row+1
        # shifted) half from a small SBUF->SBUF DMA of their own lower
        # half, so no y-shifted HBM loads are needed.  bi=0 casts go
        # first so the first matmul's operands are ready ASAP.
        dx_bases = [0, 2, 4, 6, 8, 10]
        im1c = []
        im2c = []
        # fp8 copies of the dx-compacted tiles for a=0..3, one contiguous
        # tile per image so a DoubleRow k-tile pair (a, a+1) is a simple
        # stride-750/1200 free dim.  They sit where the y staging tiles
        # used to, followed by a pad that restores the bf16 tiles' SBUF
        # offsets: the PE streams ~20% slower when the stationary and
        # moving operand tiles land on conflicting subbanks, so keep the
        # known-good absolute placement fixed.
        # the dual-fp8 LDWEIGHTS k-tile stride must be a multiple of 16
        # bytes, so the im2 fp8 tile pads each dx-base block from 750 to
        # 768 bytes (im1's 1200 is already a multiple of 16).
        S2 = 768
        n8 = len({ap for ap, _ in FP8_CELLS} | {ap + 1 for ap, _ in FP8_CELLS})
        if n8:
            im2c8 = imgs.tile([128, n8, S2], f8, name="im2c8")
            im1c8 = imgs.tile([128, n8, H, WP], f8, name="im1c8")
        pad_f32 = (10400 - n8 * (S2 + H * WP)) // 4
        imgs.tile([128, pad_f32], f32, name="pad")
        with tc.high_priority():
            for bi, dx in enumerate(dx_bases):
                c2 = imgs.tile([128, IM2_ROWS, WP], MM_DT, name=f"im2c_{bi}")
                c1 = imgs.tile([128, H, WP], MM_DT, name=f"im1c_{bi}")
                if dx < 10:
                    nc.vector.tensor_copy(c2[:], im2x[:, :, dx : dx + WP].bitcast(f32))
                    if bi < 2:
                        # scalar runs these concurrently with the im2c casts
                        # on DVE so the first matmuls' operands close early
                        nc.scalar.activation(
                            c1[:], im1x[:, :, dx : dx + WP].bitcast(f32), COPY
                        )
                    else:
                        nc.vector.tensor_copy(
                            c1[:], im1x[:, :, dx : dx + WP].bitcast(f32)
                        )
                else:
                    nc.vector.tensor_copy(
                        c2[0:C], im2x[0:C, :, dx : dx + WP].bitcast(f32)
                    )
                    nc.vector.tensor_copy(
                        c1[0:C], im1x[0:C, :, dx : dx + WP].bitcast(f32)
                    )
                    nc.gpsimd.dma_start(
                        c2[C : 2 * C, 0 : IM2_ROWS - 1, :], c2[0:C, 1:IM2_ROWS, :]
                    )
                    nc.gpsimd.dma_start(c1[C : 2 * C, 0 : H - 1, :], c1[0:C, 1:H, :])
                im2c.append(c2)
                im1c.append(c1)
                # fp8 copies trail the bf16 ones they duplicate; they are
                # only consumed after each group's ~37 bf16 matmuls.
                if n8 and bi < n8:
                    nc.vector.tensor_copy(
                        im2c8[:, bi, 0 : IM2_ROWS * WP].rearrange(
                            "p (y x) -> p y x", y=IM2_ROWS
                        ),
                        im2x[:, :, dx : dx + WP].bitcast(f32),
                    )
                    nc.scalar.activation(
                        im1c8[:, bi], im1x[:, :, dx : dx + WP].bitcast(f32), COPY
                    )

        # ---- norm DVE chain: separable 11x11 box sum of im1^2 over (y, x).
        # Shift-add log tree: widths 1->2->4->8->11.
        sq = nrm.tile([C, H, W], f32)
        nc.scalar.activation(sq[:], im1x[0:C].bitcast(f32), SQUARE)

        t2 = nrm.tile([C, H, W - 1], f32)
        nc.vector.tensor_add(t2[:], sq[:, :, 0 : W - 1], sq[:, :, 1:W])
        t4 = nrm.tile([C, H, W - 3], f32)
        nc.vector.tensor_add(t4[:], t2[:, :, 0 : W - 3], t2[:, :, 2 : W - 1])
        t8 = nrm.tile([C, H, W - 7], f32)
        nc.vector.tensor_add(t8[:], t4[:, :, 0 : W - 7], t4[:, :, 4 : W - 3])
        rp_a = nrm.tile([C, H, WP], f32)
        nc.vector.tensor_add(rp_a[:], t8[:, :, 0:WP], t2[:, :, 8 : 8 + WP])
        rp = nrm.tile([C, H, WP], f32)
        nc.vector.tensor_add(rp[:], rp_a[:], sq[:, :, 10 : 10 + WP])

        u2 = nrm.tile([C, H - 1, WP], f32)
        nc.vector.tensor_add(u2[:], rp[:, 0 : H - 1], rp[:, 1:H])
        u4 = nrm.tile([C, H - 3, WP], f32)
        nc.vector.tensor_add(u4[:], u2[:, 0 : H - 3], u2[:, 2 : H - 1])
        u8 = nrm.tile([C, H - 7, WP], f32)
        nc.vector.tensor_add(u8[:], u4[:, 0 : H - 7], u4[:, 4 : H - 3])
        nc_a = nrm.tile([C, HP, WP], f32)
        nc.vector.tensor_add(nc_a[:], u8[:, 0:HP], u2[:, 8 : 8 + HP])
        normc = nrm.tile([C, HP, WP], f32)
        nc.vector.tensor_add(normc[:], nc_a[:], rp[:, 10 : 10 + HP])

        NL = HALF * WP  # 450: l columns per l-chunk
        ncv = normc[:].rearrange("p y x -> p (y x)")
        normc_16 = nrm.tile([C, 2 * NL], f16)
        nc.vector.tensor_copy(normc_16[:], ncv)

        # ---- main correlation matmuls.  121 shifts = 60 packed pairs + 1
        # K=64 single (dy=10, dx=10).
        row_chunks = [(0, 4), (4, 4), (8, 4), (12, 3)]

        fp8_cells = set(FP8_CELLS)

        def emit_chunk_mms(r0, nr, j_order=(0, 1), fp8_late=False):
            M = nr * WP

            def emit_bf16(j):
                first = True
                for bi, dx in enumerate(dx_bases):
                    dys = range(KER) if dx < 10 else range(0, KER, 2)
                    for dy in dys:
                        if dx < 10 and ((bi // 2) * 2, dy) in fp8_cells:
                            continue  # covered by a DoubleRow cell below
                        kp = C if (dx == 10 and dy == 10) else 2 * C
                        lhsT = im2c[bi][0:kp, r0 + dy : r0 + dy + nr, :]
                        rhs = im1c[bi][0:kp, HALF * j + dy : HALF * j + dy + HALF, :]
                        last = dx == 10 and dy == 10 and not FP8_CELLS
                        nc.tensor.matmul(ps[j][0:M], lhsT, rhs, start=first, stop=last)
                        first = False

            def emit_fp8(j):
                for ci, (ap, dy) in enumerate(FP8_CELLS):
                    last = ci == len(FP8_CELLS) - 1
                    lhsT = im2c8[
                        :, ap : ap + 2, WP * (r0 + dy) : WP * (r0 + dy) + M
                    ]
                    rhs = im1c8[
                        :, ap : ap + 2, HALF * j + dy : HALF * j + dy + HALF, :
                    ]
                    nc.tensor.matmul(
                        ps[j][0:M], lhsT, rhs, start=False, stop=last, perf_mode=DR
                    )

            ps = [
                psum.tile([128, NL], f32, tag="ps", name=f"ps_{r0}_{j}")
                for j in range(2)
            ]
            if fp8_late:
                # first chunk: alternate j0/j1 per bi pair so each fresh
                # cast buys twice the PE runway (the DVE produces a
                # compacted pair every ~1.7us but one j consumes it in
                # ~1us), and all DoubleRow cells go last so the fp8
                # casts are long done.
                first = [True, True]
                for bi, dx in enumerate(dx_bases):
                    dys = range(KER) if dx < 10 else range(0, KER, 2)
                    for j in j_order:
                        for dy in dys:
                            if dx < 10 and ((bi // 2) * 2, dy) in fp8_cells:
                                continue
                            kp = C if (dx == 10 and dy == 10) else 2 * C
                            lhsT = im2c[bi][0:kp, r0 + dy : r0 + dy + nr, :]
                            rhs = im1c[bi][
                                0:kp, HALF * j + dy : HALF * j + dy + HALF, :
                            ]
                            nc.tensor.matmul(
                                ps[j][0:M], lhsT, rhs, start=first[j], stop=False
                            )
                            first[j] = False
                for j in j_order:
                    emit_fp8(j)
            else:
                for j in j_order:
                    emit_bf16(j)
                    emit_fp8(j)
            return ps

        red_all = reds.tile([128, 4], f32, name="red_all")
        nc.vector.memset(red_all[:], 0.0)

        def emit_epilogue(ci, r0, nr, ps, j_order=(0, 1)):
            # fused (psum * inv) -> running max: second j chains off the
            # first via the reduce's initial-value AP, writing the final
            # column of red_all directly.
            M = nr * WP
            ja, jb = j_order
            USE_TTR = False
            if USE_TTR:
                reda = reds.tile([128, 1], f32, tag="red", name=f"red_{r0}")
                for j, acc, init in (
                    (ja, reda, -3.0e38),
                    (jb, red_all, reda),
                ):
                    sc = scr.tile([128, NL], f32, tag="sc", name=f"sc{j}_{r0}")
                    acc_ap = (
                        acc[0:M, ci : ci + 1] if acc is red_all else acc[0:M]
                    )
                    nc.vector.tensor_tensor_reduce(
                        out=sc[0:M],
                        in0=ps[j][0:M],
                        in1=inv_bc[0:M, NL * j : NL * (j + 1)],
                        scale=1.0,
                        scalar=init if isinstance(init, float) else init[0:M],
                        op0=MULT,
                        op1=MAX,
                        accum_out=acc_ap,
                    )
                return
            red = [None, None]
            for j in j_order:
                sc = scr.tile([128, NL], f32, tag="sc", name=f"sc{j}_{r0}")
                red[j] = reds.tile([128, 1], f32, tag="red", name=f"red{j}_{r0}")
                nc.vector.tensor_tensor(
                    out=sc[0:M],
                    in0=ps[j][0:M],
                    in1=inv_bc[0:M, NL * j : NL * (j + 1)],
                    op=MULT,
                )
                nc.vector.tensor_reduce(
                    out=red[j][0:M], in_=sc[0:M], axis=mybir.AxisListType.X, op=MAX
                )
            nc.vector.tensor_tensor(
                out=red_all[0:M, ci : ci + 1], in0=red[0][0:M], in1=red[1][0:M], op=MAX
            )

        chunk_ps = {}
        chunk_ps[0] = emit_chunk_mms(*row_chunks[0], fp8_late=bool(FP8_CELLS))
        chunk_ps[1] = emit_chunk_mms(*row_chunks[1])

        # norm matmul group 1: fp16 channel sum -> sqrt (scalar engine
        # writes fp16 directly).  The whole chain runs at fp16 DVE rate;
        # 1/max(norm,1e-4) == min(1/norm, 1e4) folds the eps clamp into
        # one DVE min on the fp16 row.
        norm_16 = nrm.tile([1, 2 * NL], f16)
        inv_16 = nrm.tile([1, 2 * NL], f16)
        for j in range(2):
            nm = psum.tile([1, NL], f32, tag="ps", name=f"nm_{j}")
            sl = slice(NL * j, NL * (j + 1))
            nc.tensor.matmul(nm[:], ones_k[:], normc_16[:, sl], start=True, stop=True)
            nc.scalar.activation(norm_16[:, sl], nm[:], SQRT)

        chunk_ps[2] = emit_chunk_mms(*row_chunks[2])

        with nc.allow_low_precision(reason="1/norm only needs ~1e-3"):
            nc.vector.reciprocal(inv_16[:], norm_16[:])
            nc.vector.tensor_scalar_min(inv_16[:], inv_16[:], 1.0e4)

        inv_bc = nrm.tile([128, 2 * NL], f32)
        for j in range(2):
            ip = psum.tile([128, NL], f32, tag="ps", name=f"ip_{j}")
            sl = slice(NL * j, NL * (j + 1))
            nc.tensor.matmul(ip[:], ones_m[:], inv_16[:, sl], start=True, stop=True)
            nc.vector.tensor_copy(inv_bc[:, sl], ip[:])

        emit_epilogue(0, *row_chunks[0], chunk_ps[0])
        # last chunk: j=1 matmuls first so the final accumulation group
        # (j=0) closes last and epilogue(j=1) overlaps its stream.
        chunk_ps[3] = emit_chunk_mms(*row_chunks[3], j_order=(1, 0))
        emit_epilogue(1, *row_chunks[1], chunk_ps[1])
        emit_epilogue(2, *row_chunks[2], chunk_ps[2])
        emit_epilogue(3, *row_chunks[3], chunk_ps[3], j_order=(1, 0))
        nc.gpsimd.dma_start(out_d, red_all[:])

    nc.compile()
    return nc


def _get_program():
    global _PROGRAM
    if _PROGRAM is None:
        _PROGRAM = _build_program()
    return _PROGRAM


def make_in_maps(im1: np.ndarray, im2: np.ndarray):
    in_maps = []
    for b in range(B):
        for h in range(2):
            in_maps.append(
                {
                    "im1": np.ascontiguousarray(im1[b], dtype=np.float32),
                    "im2s": np.ascontiguousarray(
                        im2[b][:, HALF * h : HALF * h + IM2_ROWS, :], dtype=np.float32
                    ),
                }
            )
    return in_maps


ROW_CHUNKS = [(0, 4), (4, 4), (8, 4), (12, 3)]


def _half_from_cols(cols):
    half = np.empty((HALF * WP,), dtype=np.float32)
    for ci, (r0, nr) in enumerate(ROW_CHUNKS):
        half[WP * r0 : WP * r0 + nr * WP] = cols[0 : nr * WP, ci]
    return half.reshape(HALF, WP)


def assemble(results):
    out = np.empty((B, 1, HP, WP), dtype=np.float32)
    for b in range(B):
        top = _half_from_cols(results[2 * b]["out"])
        bot = _half_from_cols(results[2 * b + 1]["out"])
        out[b, 0] = np.concatenate([top, bot], axis=0)
    return out


def run(im1: np.ndarray, im2: np.ndarray, trace: bool = False):
    from concourse import bass_utils

    nc = _get_program()
    res = bass_utils.run_bass_kernel_spmd(
        nc, make_in_maps(im1, im2), core_ids=list(range(N_CORES)), trace=trace
    )
    return assemble(res.results), res


def kernel(im1: np.ndarray, im2: np.ndarray) -> np.ndarray:
    out, _ = run(np.asarray(im1), np.asarray(im2))
    return out
